# revision 43
# baseline (speedup 1.0000x reference)
"""Trainium2 Bass kernel: custom multi-head attention (seq-first, packed in-proj,
"faithful" non-standard head recombine) sharded over 8 NeuronCores.

Sharding: batch*heads across cores. B=2, H=16 -> 32 (b,h) pairs, 4 per core
(same batch per core), processed as 2 head-pair groups g of 2 heads each.

Pipeline design (v3) — ACT (exp) is the roofline at ~147us/core; everything
else is scheduled to keep the exp stream dense:
 - Attention inner stream per (g, qt, ks): one packed S^T pair (both heads
   row-packed, K=64 halves), ONE [128,1024] exp covering both heads, then a
   trailing P@V pair (col-packed M=64) plus a denominator pair (M=1
   ones-matmuls accumulated in PSUM over ks) — denominators cost no DVE work.
 - V is projected DIRECTLY in natural [T,D] layout on the PE (lhsT = x^T
   t-chunk, rhs = Wv slice, bias via rank-1 ones matmul) — no transposes.
 - Softmax denominators: per-qt PSUM row-sums are DMA-repacked [1,512] ->
   [128,4] so ONE cheap DVE reciprocal covers a whole group, then unpacked
   and rank-1-broadcast on the PE; normalization is one DVE mul per qt.
 - The PE's spare cycles inside the ACT-bound attention stream are filled by
   a deadline-paced payload queue: projections of the next group and the
   epilogue (normalize + out-proj) of the previous group are emitted
   interleaved into the attention instruction stream, so the PE never idles
   long enough for HAM to re-throttle and ACT never waits on PSUM.
 - Out-proj accumulates in 4-j-pair chunks, each into a fresh PSUM tile
   folded into an SBUF fp32 accumulator on the DVE, so no PSUM tile is held
   across interleaved attention iterations (ring would deadlock otherwise).
 - Inputs stream on both HWDGE queues (sync: x tt01 + repack + y;
   scalar/Act: bias + wqkv + x tt23 + wo2) with single-DMA packed bias.
 - PSUM: tag 'st' [128,1024]x3 (S tiles / proj groups / V-chunks / out-proj),
   tag 'op' [128,512]x1 (P@V accum), tag 'dn' [128,512]x1 (denoms / rbs)
   = 8 banks exactly.

Output mapping: the reference's out.reshape(B,T,E) maps head h's (T,D) block
to output rows h*128..h*128+127, so each core produces a disjoint slice of
the output -> no collectives. Final projection consumes the scrambled
reshape analytically via 16 column-strided lhsT slices (j = query mod 16)
against natural + half-swapped copies of out_w (prepared on host).
"""

import numpy as np
from collections import deque
from contextlib import ExitStack

import concourse.bass as bass
import concourse.mybir as mybir
import concourse.tile as tile
from concourse import bacc
from concourse.bass_utils import run_bass_kernel_spmd

F16 = mybir.dt.float16
F32 = mybir.dt.float32

B, E, H, D = 2, 1024, 16, 64
NCORES = 8
SCALE = D ** -0.5
P = 128


def build_nc(T=2048, debug=False, reps=1):
    n_ks = T // P            # 16 key slices of 128
    n_qt = T // 512          # 4 query tiles
    QW = 512
    n_es = E // P            # 8 contraction slices
    n_tt = T // 512          # 4 t-tiles for projections
    TW = 512
    R = T // 16              # 128 output rows per head block
    assert T == 2048, "schedule is tuned for T=2048"

    nc = bacc.Bacc("TRN2", target_bir_lowering=False, debug=False)

    xP = nc.dram_tensor("xP", [P, E // P, T], F16, kind="ExternalInput").ap()
    wq3 = nc.dram_tensor(
        "wq3", [3, P, 2, E // P, 128], F16, kind="ExternalInput"
    ).ap()
    bqkv_p = nc.dram_tensor("bqkv_p", [P, 2, 3], F32, kind="ExternalInput").ap()
    bvb = nc.dram_tensor("bvb", [P, 2, P], F32, kind="ExternalInput").ap()
    wo2 = nc.dram_tensor("wo2", [2, 8, 128, E], F16, kind="ExternalInput").ap()
    ob = nc.dram_tensor("ob", [E], F16, kind="ExternalInput").ap()
    y = nc.dram_tensor("y", [4, R, E], F32, kind="ExternalOutput").ap()
    if debug:
        dbg_qkv = nc.dram_tensor(
            "dbg_qkv", [2, P, 2, T], F16, kind="ExternalOutput"
        ).ap()
        dbg_outT = nc.dram_tensor(
            "dbg_outT", [P, 2, T], F16, kind="ExternalOutput"
        ).ap()
        dbg_vnat = nc.dram_tensor(
            "dbg_vnat", [P, 2, 2, n_ks, D], F16, kind="ExternalOutput"
        ).ap()
        dbg_dn = nc.dram_tensor(
            "dbg_dn", [33, 2, n_qt, QW], F32, kind="ExternalOutput"
        ).ap()

    with tile.TileContext(nc) as tc, ExitStack() as ctx:
        consts = ctx.enter_context(tc.tile_pool(name="consts", bufs=1))
        sb_w = ctx.enter_context(tc.tile_pool(name="sb_w", bufs=1))
        sb_x = ctx.enter_context(tc.tile_pool(name="sb_x", bufs=1))
        sb_qkv = ctx.enter_context(tc.tile_pool(name="sb_qkv", bufs=1))
        sb_wo = ctx.enter_context(tc.tile_pool(name="sb_wo", bufs=1))
        sb_nrm = ctx.enter_context(tc.tile_pool(name="sb_nrm", bufs=1))
        ring = ctx.enter_context(tc.tile_pool(name="ring", bufs=16))
        psum = ctx.enter_context(tc.tile_pool(name="ps", bufs=1, space="PSUM"))

        # ---- constants & weights ----
        ones_t = consts.tile([P, P], F16, tag="ones")
        nc.vector.memset(ones_t[:], 1.0)
        ob_sb = consts.tile([1, E], F16, tag="ob")
        bias_sb = consts.tile([P, 2, 3], F32, tag="bias")
        nc.scalar.dma_start(bias_sb[:], bqkv_p)
        bvb_sb = consts.tile([P, 2, P], F32, tag="bvb")
        nc.scalar.dma_start(bvb_sb[:], bvb)

        # x + wqkv pre-laid-out on host -> contiguous fat DMAs, split across
        # both HWDGE queues. One SBUF tile per tt so Tile's dependency
        # tracking is exact (a single tile made early matmuls wait on later
        # tt DMAs); tt0 halves first so projections start ~11us in.
        x_tt = [
            sb_x.tile([P, n_es, TW], F16, tag=f"xT{tt}", name=f"x{tt}")
            for tt in range(n_tt)
        ]
        w_sb = sb_w.tile([P, 3, 2, n_es, P], F16, tag="wqkv")

        nc.sync.dma_start(x_tt[0][:, 0:4, :], xP[:, 0:4, 0:TW])
        nc.scalar.dma_start(w_sb[:, 1], wq3[1])          # K weights
        nc.scalar.dma_start(x_tt[0][:, 4:8, :], xP[:, 4:8, 0:TW])
        nc.scalar.dma_start(w_sb[:, 0], wq3[0])          # Q
        nc.scalar.dma_start(w_sb[:, 2], wq3[2])          # V
        for tt in range(1, n_tt):
            nc.sync.dma_start(
                x_tt[tt][:], xP[:, :, tt * TW:(tt + 1) * TW]
            )

        wo_sb = sb_wo.tile([P, 2, 8, E], F16, tag="wo")

        qt_sb = sb_qkv.tile([P, 2, T], F16, tag="QT")
        kt_sb = sb_qkv.tile([P, 2, T], F16, tag="KT")
        vnat = sb_qkv.tile([P, 2, 2, n_ks, D], F16, tag="Vnat")
        outT = sb_qkv.tile([P, 2, T], F16, tag="outT")
        ops_sb = sb_nrm.tile([P, 2, n_qt, QW], F32, tag="ops")
        dn_sb = sb_nrm.tile([33, 2, n_qt, QW], F32, tag="dnsb")
        dn_t = sb_nrm.tile([P, 2, 2, n_qt, n_qt], F32, tag="dnt")
        rcp_t = sb_nrm.tile([P, 2, 2, n_qt, n_qt], F32, tag="rcpt")
        rcp16_t = sb_nrm.tile([P, 2, 2, n_qt, n_qt], F16, tag="rcpt16")
        rcp16 = sb_nrm.tile([33, 2, n_qt, QW], F16, tag="rcp16")

        dest_of = {0: qt_sb, 1: kt_sb}

        def proj_groups(pairs):
            # emit 1-2 [128,512] K/Q projection groups sharing one 'st' tile
            def th():
                ps_tile = psum.tile([P, 2 * TW], F32, tag="st", bufs=3,
                                    name="pr")
                for half, (g, t, tt) in enumerate(pairs):
                    lo = half * TW
                    for es in range(n_es):
                        nc.tensor.matmul(
                            ps_tile[:, lo:lo + TW],
                            lhsT=w_sb[:, t, g, es, :],
                            rhs=x_tt[tt][:, es, :],
                            start=(es == 0),
                            stop=(es == n_es - 1),
                        )
                    nc.vector.tensor_scalar_add(
                        dest_of[t][:, g, tt * TW:(tt + 1) * TW],
                        ps_tile[:, lo:lo + TW],
                        bias_sb[:, g, t].unsqueeze(1),
                    )
            return th

        def v_chunk(g, ks0):
            # V in natural layout: 2 t-chunks (ks0, ks0+1), both heads.
            # out[t,d] = sum_es x_sb[:,es,t-chunk].T @ w_sb[:,g,2,es,:] + bv
            def th():
                ps_tile = psum.tile([P, 2 * TW], F32, tag="st", bufs=3,
                                    name="vp")
                for i, ks in enumerate((ks0, ks0 + 1)):
                    lo = i * P  # [128,128] fp32 = 512B, within first bank
                    for es in range(n_es):
                        ko = (ks % 4) * P
                        nc.tensor.matmul(
                            ps_tile[:, lo:lo + P],
                            lhsT=x_tt[ks // 4][:, es, ko:ko + P],
                            rhs=w_sb[:, 2, g, es, :],
                            start=(es == 0), stop=(es == n_es - 1),
                        )
                    # bias folded into the evacuating DVE add (bvb is the
                    # host-broadcast V bias, [128 x 128] per g)
                    nc.vector.tensor_add(
                        vnat[:, g, :, ks, :], ps_tile[:, lo:lo + P],
                        bvb_sb[:, g, :],
                    )
            return th

        # ---- attention machinery ----
        state = {}

        def attn_s_exp(g, qt, ks):
            q0 = qt * QW
            st = psum.tile([P, 2 * QW], F32, tag="st", bufs=3, name="st")
            nc.tensor.matmul(
                st[:, 0:QW],
                lhsT=kt_sb[0:64, g, ks * P:(ks + 1) * P],
                rhs=qt_sb[0:64, g, q0:q0 + QW],
                start=True, stop=True, tile_position=(0, 0),
            )
            nc.tensor.matmul(
                st[:, QW:2 * QW],
                lhsT=kt_sb[64:128, g, ks * P:(ks + 1) * P],
                rhs=qt_sb[64:128, g, q0:q0 + QW],
                start=True, stop=True, tile_position=(64, 0),
            )
            pt = ring.tile([P, 2 * QW], F16, tag="pt", name="pt")
            nc.scalar.activation(
                pt[:], st[:], mybir.ActivationFunctionType.Exp, scale=SCALE
            )
            return pt

        def attn_pv(g, qt, ks_pt_list):
            # P@V for a pair of key slices, emitted back-to-back so the
            # continuing accumulation groups pipeline their weight loads
            first = ks_pt_list[0][0] == 0
            last = ks_pt_list[-1][0] == n_ks - 1
            if first:
                state[(g, qt, "op")] = psum.tile(
                    [P, QW], F32, tag="op", bufs=1, name="op"
                )
            op = state[(g, qt, "op")]
            for ks, pt in ks_pt_list:
                f, l = (ks == 0), (ks == n_ks - 1)
                nc.tensor.matmul(
                    op[0:64, :], lhsT=vnat[:, g, 0, ks, :], rhs=pt[:, 0:QW],
                    start=f, stop=l, tile_position=(0, 0),
                    skip_group_check=True,
                )
                nc.tensor.matmul(
                    op[64:128, :], lhsT=vnat[:, g, 1, ks, :],
                    rhs=pt[:, QW:2 * QW],
                    start=f, stop=l, tile_position=(0, 64),
                    skip_group_check=True,
                )
            if last:
                nc.vector.tensor_copy(
                    ops_sb[:, g, qt, :], state.pop((g, qt, "op"))[:]
                )

        def attn_dn(g, qt, ks_pt_list):
            # lazily allocate the dn tile so its 'dn'-ring slot follows any
            # rbs tiles of the previous group (emission order = ring order)
            first = ks_pt_list[0][0] == 0
            last = ks_pt_list[-1][0] == n_ks - 1
            if first:
                state[(g, qt, "dn")] = psum.tile(
                    [P, QW], F32, tag="dn", bufs=1, name="dn"
                )
            dn = state[(g, qt, "dn")]
            for ks, pt in ks_pt_list:
                f, l = (ks == 0), (ks == n_ks - 1)
                nc.tensor.matmul(
                    dn[0:1, :], lhsT=ones_t[:, 0:1], rhs=pt[:, 0:QW],
                    start=f, stop=l, tile_position=(0, 0),
                    skip_group_check=True,
                )
                nc.tensor.matmul(
                    dn[32:33, :], lhsT=ones_t[:, 0:1], rhs=pt[:, QW:2 * QW],
                    start=f, stop=l, tile_position=(0, 32),
                    skip_group_check=True,
                )
            if last:
                nc.vector.tensor_copy(
                    dn_sb[:, g, qt, :], state.pop((g, qt, "dn"))[0:33, :]
                )
                recip_roundtrip(g, qt)

        def recip_roundtrip(g, qt):
            # [1,512] denominator rows -> [128,4] columns, one reciprocal,
            # cast, and back — per qt, so the chain overlaps the next qt's
            # attention instead of serializing at the end of the group.
            for half in range(2):
                nc.sync.dma_start(
                    dn_t[:, g, half, qt, :],
                    dn_sb[32 * half:32 * half + 1, g, qt, :],
                )
            nc.vector.reciprocal(rcp_t[:, g, :, qt, :], dn_t[:, g, :, qt, :])
            nc.vector.tensor_copy(
                rcp16_t[:, g, :, qt, :], rcp_t[:, g, :, qt, :]
            )
            for half in range(2):
                nc.sync.dma_start(
                    rcp16[32 * half:32 * half + 1, g, qt, :],
                    rcp16_t[:, g, half, qt, :],
                )

        def rbs_mul(g, qt):
            q0 = qt * QW
            rbs = psum.tile([P, QW], F32, tag="dn", bufs=1, name="rbs")
            nc.tensor.matmul(
                rbs[0:64, :], lhsT=ones_t[0:1, 0:64],
                rhs=rcp16[0:1, g, qt, :],
                start=True, stop=True, tile_position=(0, 0),
                skip_group_check=True,
            )
            nc.tensor.matmul(
                rbs[64:128, :], lhsT=ones_t[32:33, 0:64],
                rhs=rcp16[32:33, g, qt, :],
                start=True, stop=True, tile_position=(32, 64),
                skip_group_check=True,
            )
            nc.vector.tensor_mul(
                outT[:, g, q0:q0 + QW], ops_sb[:, g, qt, :], rbs[:]
            )

        def outproj_monolithic(g):
            # tail version: PSUM is free, hold one tile per ft for all 16 j
            oT = outT[:, g, :].rearrange("p (r j) -> p j r", j=16)
            for ft in range(E // 512):
                f0 = ft * 512
                yp = psum.tile([P, 2 * TW], F32, tag="st", bufs=3, name="yp")
                for j in range(16):
                    s = j // 2
                    ca = 0 if j % 2 == 0 else 1
                    cb = 1 - ca
                    nc.tensor.matmul(
                        yp[0:R, 0:TW], lhsT=oT[0:64, j, :],
                        rhs=wo_sb[0:64, ca, s, f0:f0 + 512],
                        start=(j == 0), stop=False, tile_position=(0, 0),
                    )
                    nc.tensor.matmul(
                        yp[0:R, TW:2 * TW], lhsT=oT[64:128, j, :],
                        rhs=wo_sb[64:128, cb, s, f0:f0 + 512],
                        start=(j == 0), stop=False, tile_position=(64, 0),
                    )
                nc.tensor.matmul(
                    yp[0:R, 0:TW], lhsT=ones_t[0:1, 0:R],
                    rhs=ob_sb[0:1, f0:f0 + 512],
                    start=False, stop=True, tile_position=(0, 0),
                )
                nc.tensor.matmul(
                    yp[0:R, TW:2 * TW], lhsT=ones_t[0:1, 0:R],
                    rhs=ob_sb[0:1, f0:f0 + 512],
                    start=False, stop=True, tile_position=(0, 0),
                )
                ys = sb_nrm.tile(
                    [P, 2 * TW], F32, tag="ystage", bufs=2, name="ys"
                )
                nc.vector.tensor_copy(ys[0:R, :], yp[0:R, :])
                nc.sync.dma_start(y[2 * g + 0, :, f0:f0 + 512], ys[0:R, 0:TW])
                nc.sync.dma_start(
                    y[2 * g + 1, :, f0:f0 + 512], ys[0:R, TW:2 * TW]
                )

        def make_outproj_thunks(g):
            # per ft: 4 chunk-thunks, each a fresh PSUM tile of 4 j-pairs
            # folded into an SBUF accumulator (no tile held across thunks)
            oT = outT[:, g, :].rearrange("p (r j) -> p j r", j=16)
            thunks = []
            accs = {}

            def chunk(ft, jc):
                def th():
                    f0 = ft * 512
                    yp = psum.tile(
                        [P, 2 * TW], F32, tag="st", bufs=3, name="yp"
                    )
                    last = (jc == 3)
                    for i, j in enumerate(range(4 * jc, 4 * jc + 4)):
                        s = j // 2
                        ca = 0 if j % 2 == 0 else 1
                        cb = 1 - ca
                        stop_f = (i == 3) and not last
                        nc.tensor.matmul(
                            yp[0:R, 0:TW],
                            lhsT=oT[0:64, j, :],
                            rhs=wo_sb[0:64, ca, s, f0:f0 + 512],
                            start=(i == 0), stop=stop_f,
                            tile_position=(0, 0),
                        )
                        nc.tensor.matmul(
                            yp[0:R, TW:2 * TW],
                            lhsT=oT[64:128, j, :],
                            rhs=wo_sb[64:128, cb, s, f0:f0 + 512],
                            start=(i == 0), stop=stop_f,
                            tile_position=(64, 0),
                        )
                    if last:
                        nc.tensor.matmul(
                            yp[0:R, 0:TW], lhsT=ones_t[0:1, 0:R],
                            rhs=ob_sb[0:1, f0:f0 + 512],
                            start=False, stop=True, tile_position=(0, 0),
                        )
                        nc.tensor.matmul(
                            yp[0:R, TW:2 * TW], lhsT=ones_t[0:1, 0:R],
                            rhs=ob_sb[0:1, f0:f0 + 512],
                            start=False, stop=True, tile_position=(0, 0),
                        )
                    if jc == 0:
                        accs[ft] = sb_nrm.tile(
                            [P, 2 * TW], F32, tag="ystage", bufs=2, name="ys"
                        )
                        nc.vector.tensor_copy(accs[ft][0:R, :], yp[0:R, :])
                    else:
                        nc.vector.tensor_add(
                            accs[ft][0:R, :], accs[ft][0:R, :], yp[0:R, :]
                        )
                    if last:
                        ys = accs.pop(ft)
                        nc.sync.dma_start(
                            y[2 * g + 0, :, f0:f0 + 512], ys[0:R, 0:TW]
                        )
                        nc.sync.dma_start(
                            y[2 * g + 1, :, f0:f0 + 512], ys[0:R, TW:2 * TW]
                        )
                return th

            for ft in range(E // 512):
                for jc in range(4):
                    thunks.append(chunk(ft, jc))
            return thunks

        # ---- PE clock warmup: ~6us of dependency-free matmuls while the
        # input DMAs stream, so the ramp projections run at K=8/8 ----
        warm = psum.tile([P, 2 * TW], F32, tag="st", bufs=3, name="warm")
        for i in range(28):
            nc.tensor.matmul(
                warm[:, 0:P], lhsT=ones_t[:], rhs=ones_t[:],
                start=(i == 0), stop=(i == 27),
            )

        # ---- main emission ----
        for _rep in range(reps):
            wo_loaded = _rep > 0
            for g in range(2):
                # payload: list of (due_iter, thunk), popped when it >= due
                payload = []
                dn_pending = deque()
                dn_state = {"open": g == 0}
                if g == 0:
                    # ramp: K/Q tt0 and V ks0..3 of g0 emitted up front
                    proj_groups([(0, 1, 0), (0, 0, 0)])()
                    v_chunk(0, 0)()
                    v_chunk(0, 2)()
                    payload = [
                        (1, v_chunk(0, 4)),
                        (2, proj_groups([(0, 1, 1)])),
                        (3, v_chunk(0, 6)),
                        (5, v_chunk(0, 8)),
                        (6, proj_groups([(0, 1, 2)])),
                        (7, v_chunk(0, 10)),
                        (9, v_chunk(0, 12)),
                        (10, proj_groups([(0, 1, 3)])),
                        (11, v_chunk(0, 14)),
                        (13, proj_groups([(0, 0, 1)])),
                        (17, proj_groups([(0, 0, 2), (0, 0, 3)])),
                        (22, proj_groups([(1, 1, 0), (1, 0, 0)])),
                        (25, proj_groups([(1, 1, 1)])),
                        (28, v_chunk(1, 0)),
                        (31, v_chunk(1, 2)),
                        (34, v_chunk(1, 4)),
                        (37, v_chunk(1, 6)),
                        (40, proj_groups([(1, 1, 2)])),
                        (43, proj_groups([(1, 1, 3)])),
                    ]
                    if not wo_loaded:
                        def load_wo():
                            nc.scalar.dma_start(
                                wo_sb[:], wo2.rearrange("c s p f -> p c s f")
                            )
                            nc.scalar.dma_start(ob_sb[:], ob.unsqueeze(0))
                        payload.append((15, load_wo))
                else:
                    def open_dn():
                        dn_state["open"] = True
                        while dn_pending:
                            dn_pending.popleft()()
                    opj = make_outproj_thunks(0)
                    payload = [
                        (2, lambda: rbs_mul(0, 0)),
                        (3, lambda: rbs_mul(0, 1)),
                        (4, lambda: rbs_mul(0, 2)),
                        (5, lambda: rbs_mul(0, 3)),
                        (6, open_dn),
                        (7, v_chunk(1, 8)),
                        (8, v_chunk(1, 10)),
                        (9, v_chunk(1, 12)),
                        (10, v_chunk(1, 14)),
                        (12, proj_groups([(1, 0, 1)])),
                        (15, opj[0]), (18, opj[1]), (21, opj[2]),
                        (24, opj[3]),
                        (27, proj_groups([(1, 0, 2), (1, 0, 3)])),
                        (30, opj[4]), (33, opj[5]), (36, opj[6]),
                        (39, opj[7]),
                    ]

                payload.sort(key=lambda x: x[0])
                pq = deque(payload)
                it = 0
                prev = None  # trailing PV/dn one 2-ks super behind

                def do_prev():
                    pqt, lst = prev
                    attn_pv(g, pqt, lst)
                    if dn_state["open"]:
                        attn_dn(g, pqt, lst)
                    else:
                        dn_pending.append(
                            lambda a=pqt, b=list(lst): attn_dn(g, a, b)
                        )

                for qt in range(n_qt):
                    for ks2 in range(0, n_ks, 2):
                        pt_a = attn_s_exp(g, qt, ks2)
                        pt_b = attn_s_exp(g, qt, ks2 + 1)
                        if prev is not None:
                            do_prev()
                        prev = (qt, [(ks2, pt_a), (ks2 + 1, pt_b)])
                        it += 2
                        while pq and pq[0][0] <= it:
                            pq.popleft()[1]()
                do_prev()
                while pq:
                    pq.popleft()[1]()

            # tail: epilogue of g1 fully exposed
            for qt in range(n_qt):
                rbs_mul(1, qt)
            outproj_monolithic(1)

            if debug:
                nc.sync.dma_start(dbg_qkv[0], qt_sb[:])
                nc.sync.dma_start(dbg_qkv[1], kt_sb[:])
                nc.sync.dma_start(dbg_outT[:], outT[:])
                nc.sync.dma_start(dbg_vnat[:], vnat[:])
                nc.sync.dma_start(dbg_dn[:], dn_sb[:])

    nc.compile()
    return nc


_NC_CACHE = {}


def _get_nc(T=2048):
    if T not in _NC_CACHE:
        _NC_CACHE[T] = build_nc(T)
    return _NC_CACHE[T]


def make_in_maps(x, in_proj_w, in_proj_b, out_w, out_b):
    """Host-side sharding/layout prep -> per-core input maps."""
    T, Bx, Ex = x.shape
    x = np.asarray(x, np.float32)
    in_proj_w = np.asarray(in_proj_w, np.float32)
    in_proj_b = np.asarray(in_proj_b, np.float32)
    out_w = np.asarray(out_w, np.float32)
    out_b = np.asarray(out_b, np.float32)

    wo_nat = out_w.reshape(8, 128, Ex)
    wo_swp = np.concatenate(
        [wo_nat[:, 64:128, :], wo_nat[:, 0:64, :]], axis=1
    )
    wo2 = np.stack([wo_nat, wo_swp], axis=0).astype(np.float16)
    ob = out_b.astype(np.float16)

    in_maps = []
    for c in range(NCORES):
        b = c // 4
        h0 = 4 * (c % 4)
        xT_m = x[:, b, :].T.astype(np.float16)          # [E, T]
        xP = np.ascontiguousarray(
            xT_m.reshape(E // 128, 128, T).transpose(1, 0, 2))
        wq, bq = [], []
        for g in range(2):
            ha = h0 + 2 * g
            cols = slice(ha * D, ha * D + 128)
            wq.append(np.stack(
                [in_proj_w[:, cols],
                 in_proj_w[:, Ex + ha * D: Ex + ha * D + 128],
                 in_proj_w[:, 2 * Ex + ha * D: 2 * Ex + ha * D + 128]],
                axis=0))
            bq.append(np.stack(
                [in_proj_b[cols],
                 in_proj_b[Ex + ha * D: Ex + ha * D + 128],
                 in_proj_b[2 * Ex + ha * D: 2 * Ex + ha * D + 128]], axis=0))
        bq_a = np.stack(bq, axis=0)                    # [2, 3, 128]
        bvb = np.ascontiguousarray(
            np.broadcast_to(bq_a[None, :, 2, :], (128, 2, 128))
        ).astype(np.float32)
        wq_a = np.stack(wq, axis=0).astype(np.float16)  # [2, 3, E, 128]
        wq3 = np.ascontiguousarray(
            wq_a.reshape(2, 3, E // 128, 128, 128).transpose(1, 3, 0, 2, 4))
        in_maps.append({
            "xP": xP,
            "wq3": wq3,
            "bqkv_p": np.ascontiguousarray(
                bq_a.transpose(2, 0, 1)).astype(np.float32),
            "bvb": bvb,
            "wo2": wo2,
            "ob": ob,
        })
    return in_maps


def assemble(results, T, Ex):
    R = T // 16
    yf = np.empty((B, T, Ex), np.float32)
    for c in range(NCORES):
        b = c // 4
        h0 = 4 * (c % 4)
        blk = results[c]["y"]
        for i in range(4):
            h = h0 + i
            yf[b, h * R:(h + 1) * R, :] = blk[i]
    return yf


def kernel(x, in_proj_w, in_proj_b, out_w, out_b):
    T = x.shape[0]
    nc = _get_nc(T)
    in_maps = make_in_maps(x, in_proj_w, in_proj_b, out_w, out_b)
    last_err = None
    for _attempt in range(3):
        try:
            res = run_bass_kernel_spmd(nc, in_maps, core_ids=list(range(NCORES)))
            return assemble(res.results, T, x.shape[2])
        except Exception as e:  # transient NRT device wedge on first touch
            last_err = e
            import time as _time
            _time.sleep(5)
    raise last_err
